# revision 12
# baseline (speedup 1.0000x reference)
"""MoE top-2 routing kernel for 8 Trainium2 NeuronCores.

Strategy (expert parallelism, per sharding hint):
  - Host computes the cheap gate (N x D @ D x E = 25 MFLOP), top-2 routing,
    combine weights and the balance loss. This *is* the sharding step: it
    decides which tokens go to which core.
  - Core e receives the tokens routed to expert e (gathered, transposed to
    [D, cap], zero-padded to a common capacity), plus expert e's weights
    pre-transposed on host so all matmul operands are K-major.
  - Device computes  y = (relu(x @ Wfc^T)^2 @ Wproj^T) * combine_weight
    with float32r matmuls (full fp32 data, 1 cycle/row at N>=256).
  - Host scatter-adds the per-expert outputs back into [N, D].

All shapes hardcoded for B=2, T=2048, D=768, H=3072, E=8, top-2.
"""

import os
import sys

for _p in ("/opt/trn_rl_repo", "/root/.axon_site/_ro/trn_rl_repo"):
    if os.path.isdir(_p) and _p not in sys.path:
        sys.path.append(_p)

from contextlib import ExitStack

import numpy as np

import concourse.bass as bass
import concourse.tile as tile
from concourse import bacc
from concourse import mybir
from concourse.bass_utils import run_bass_kernel_spmd

TOPK = 2
D, H, E = 768, 3072, 8
P = 128
KD, KH = D // P, H // P  # 6, 24
CHUNK = 256  # tokens per chunk (2 PSUM c-subtiles of 128)
F32 = mybir.dt.float32
F32R = mybir.dt.float32r
ACT = mybir.ActivationFunctionType


def _r(ap):
    # float32r: same fp32 bytes, PE streams 4B/cycle -> 4x faster than float32
    return ap.bitcast(F32R)


def build_nc(cap: int, reps: int = 1, loop_iters: int | None = None) -> bass.Bass:
    """One expert's MLP over `cap` tokens (zero-padded).

    Inputs (per core): xt [D, cap] = tokens^T; wfct [D, H] = Wfc^T;
    wprojt [H, D] = Wproj^T; scale [P, cap//P] = combine weights, token-major
    (token t = cs*128 + p lives at scale[p, cs]).
    Output: y [cap, D], already scaled by the combine weight.
    """
    assert cap % CHUNK == 0
    nc = bacc.Bacc(None, target_bir_lowering=False)
    xt = nc.dram_tensor("xt", [D, cap], F32R, kind="ExternalInput")
    wfct = nc.dram_tensor("wfct", [D, H], F32R, kind="ExternalInput")
    wprojt = nc.dram_tensor("wprojt", [H, D], F32R, kind="ExternalInput")
    scale = nc.dram_tensor("scale", [P, cap // P], F32, kind="ExternalInput")
    y = nc.dram_tensor("y", [cap, D], F32, kind="ExternalOutput")

    xt_r = xt.rearrange("(kd p) c -> p kd c", p=P)  # [128, 6, cap]
    wfct_r = wfct.rearrange("(kd p) h -> p kd h", p=P)  # [128, 6, 3072]
    wprojt_r = wprojt.rearrange("(kh p) d -> p kh d", p=P)  # [128, 24, 768]
    y_r = y.rearrange("(ct p) d -> p ct d", p=P)  # [128, cap//128, 768]

    nchunks = cap // CHUNK
    NSUB = CHUNK // P  # 2

    with ExitStack() as ctx:
        tc = ctx.enter_context(tile.TileContext(nc))
        wpool = ctx.enter_context(tc.tile_pool(name="w", bufs=1))
        xpool = ctx.enter_context(tc.tile_pool(name="x", bufs=2))
        rpool = ctx.enter_context(tc.tile_pool(name="r", bufs=4))
        apool = ctx.enter_context(tc.tile_pool(name="a", bufs=6))
        ypool = ctx.enter_context(tc.tile_pool(name="yo", bufs=3))
        ps1pool = ctx.enter_context(tc.tile_pool(name="ps1", bufs=2, space="PSUM"))
        ps2pool = ctx.enter_context(tc.tile_pool(name="ps2", bufs=1, space="PSUM"))

        # Weights resident in SBUF for the whole kernel (~144 KB/partition).
        # wfct loaded in column blocks so chunk-0 mm1 starts after ~2.4MB.
        wfct_sb = []
        for k in range(KD):
            t = wpool.tile([P, H], F32R, name=f"wfc{k}", tag=f"wfc{k}")
            wfct_sb.append(t)
        HBLK = H // 4
        for hb in range(4):
            for k in range(KD):
                nc.sync.dma_start(
                    out=wfct_sb[k][:, hb * HBLK : (hb + 1) * HBLK],
                    in_=wfct_r[:, k, hb * HBLK : (hb + 1) * HBLK],
                )
        wprojt_sb = []
        for k in range(KH):
            t = wpool.tile([P, D], F32R, name=f"wpj{k}", tag=f"wpj{k}")
            nc.sync.dma_start(out=t, in_=wprojt_r[:, k, :])
            wprojt_sb.append(t)
        scale_sb = wpool.tile([P, cap // P], F32, name="scale", tag="scale")
        nc.sync.dma_start(out=scale_sb, in_=scale[:, :])

        def full_body():
          for _rep in range(reps):
            body_once()

        def body_once():
          for ci in range(nchunks):
            xt_sb = xpool.tile([P, KD, CHUNK], F32R, name="xt", tag="xt")
            nc.sync.dma_start(
                out=xt_sb, in_=xt_r[:, :, ci * CHUNK : (ci + 1) * CHUNK]
            )

            # mm2 accumulators for this chunk: [128 tokens, 768] each (2 banks)
            ps2_tiles = [
                ps2pool.tile([P, D], F32, name=f"ps2_{cs}", tag=f"ps2_{cs}") for cs in range(NSUB)
            ]
            a_tiles = [None] * KH

            def mm1_step(hk, xt_sb=xt_sb, a_tiles=a_tiles):
                # A^T[hk] = relu(Wfc x)^2 for 128 h-rows x CHUNK tokens
                ps1 = ps1pool.tile([P, CHUNK], F32, name="ps1", tag="ps1")
                for dk in range(KD):
                    nc.tensor.matmul(
                        ps1,
                        lhsT=wfct_sb[dk][:, hk * P : (hk + 1) * P],
                        rhs=xt_sb[:, dk, :],
                        start=(dk == 0),
                        stop=(dk == KD - 1),
                    )
                r_sb = rpool.tile([P, CHUNK], F32, name="r", tag="r")
                nc.scalar.activation(out=r_sb, in_=ps1, func=ACT.Relu)
                a_sb = apool.tile([P, CHUNK], F32R, name="a", tag="a")
                nc.vector.tensor_mul(a_sb, r_sb, r_sb)
                a_tiles[hk] = a_sb

            def mm2_step(hk, ps2_tiles=ps2_tiles, a_tiles=a_tiles):
                for cs in range(NSUB):
                    a = a_tiles[hk][:, cs * P : (cs + 1) * P]
                    nc.tensor.matmul(
                        ps2_tiles[cs][:, 0:512],
                        lhsT=a,
                        rhs=wprojt_sb[hk][:, 0:512],
                        start=(hk == 0),
                        stop=(hk == KH - 1),
                    )
                    nc.tensor.matmul(
                        ps2_tiles[cs][:, 512:768],
                        lhsT=a,
                        rhs=wprojt_sb[hk][:, 512:768],
                        start=(hk == 0),
                        stop=(hk == KH - 1),
                    )

            # software pipeline: mm2 consumes a-tiles PD steps behind mm1 so
            # the PE never stalls on the relu->square (ACT->DVE) chain
            PD = 2
            for hk in range(PD):
                mm1_step(hk)
            for hk in range(PD, KH):
                mm1_step(hk)
                mm2_step(hk - PD)
            for hk in range(KH - PD, KH):
                mm2_step(hk)

            for cs in range(NSUB):
                ct = ci * NSUB + cs
                y_sb = ypool.tile([P, D], F32, name="y", tag="y")
                nc.scalar.activation(
                    out=y_sb,
                    in_=ps2_tiles[cs],
                    func=ACT.Copy,
                    scale=scale_sb[:, ct : ct + 1],
                )
                nc.sync.dma_start(out=y_r[:, ct, :], in_=y_sb)

        if loop_iters is None:
            full_body()
        else:
            with tc.For_i(0, loop_iters, 1):
                full_body()

    nc.finalize()
    return nc


def route(x, gate_w):
    """Host-side gate: returns per-expert token indices + combine weights,
    and the balance loss (matches reference.py's math)."""
    N = x.shape[0]
    logits = x @ gate_w.T  # [N, E]
    m = logits.max(axis=1, keepdims=True)
    p = np.exp(logits - m)
    probs = p / p.sum(axis=1, keepdims=True)  # [N, E]
    # top-2, ties -> lower index first (matches jax.lax.top_k)
    idx = np.argsort(-probs, axis=1, kind="stable")[:, :TOPK]  # [N, 2]
    rw = np.take_along_axis(probs, idx, axis=1)
    rw = rw / rw.sum(axis=1, keepdims=True)

    counts = np.zeros(E, np.int64)
    for k in range(TOPK):
        counts += np.bincount(idx[:, k], minlength=E)
    balance = np.float32(
        (probs.mean(axis=0).astype(np.float64) * (counts / N)).sum() * E
    )

    per_expert = []
    for e in range(E):
        sel = idx == e  # [N, 2]
        tok = np.nonzero(sel.any(axis=1))[0]
        # weight for token n at expert e: rw[n, k] where idx[n, k] == e
        w = np.where(idx[tok, 0] == e, rw[tok, 0], rw[tok, 1]).astype(np.float32)
        per_expert.append((tok, w))
    return per_expert, balance


def _prep_core_inputs(xf, w_fc, w_proj, per_expert, cap):
    in_maps = []
    for e in range(E):
        tok, w = per_expert[e]
        cnt = len(tok)
        xt = np.zeros((D, cap), np.float32)
        xt[:, :cnt] = xf[tok].T
        scale = np.zeros((P, cap // P), np.float32)
        sflat = np.zeros(cap, np.float32)
        sflat[:cnt] = w
        scale[:, :] = sflat.reshape(cap // P, P).T
        in_maps.append(
            {
                "xt": xt,
                "wfct": np.ascontiguousarray(w_fc[e].T),
                "wprojt": np.ascontiguousarray(w_proj[e].T),
                "scale": scale,
            }
        )
    return in_maps


def kernel(x, gate_w, w_fc, w_proj):
    x = np.asarray(x, np.float32)
    gate_w = np.asarray(gate_w, np.float32)
    w_fc = np.asarray(w_fc, np.float32)
    w_proj = np.asarray(w_proj, np.float32)

    B, T, _ = x.shape
    N = B * T
    xf = x.reshape(N, D)

    per_expert, balance = route(xf, gate_w)
    max_cnt = max(len(tok) for tok, _ in per_expert)
    cap = max(CHUNK, ((max_cnt + CHUNK - 1) // CHUNK) * CHUNK)

    nc = build_nc(cap)
    in_maps = _prep_core_inputs(xf, w_fc, w_proj, per_expert, cap)
    results = run_bass_kernel_spmd(nc, in_maps, core_ids=list(range(E))).results

    out = np.zeros((N, D), np.float32)
    for e in range(E):
        tok, _ = per_expert[e]
        out[tok] += results[e]["y"][: len(tok)]
    return out.reshape(B, T, D), balance


# revision 15
# speedup vs baseline: 1.1557x; 1.1557x over previous
"""MoE top-2 routing kernel for 8 Trainium2 NeuronCores.

Strategy (expert parallelism, per sharding hint):
  - Host computes the cheap gate (N x D @ D x E = 25 MFLOP), top-2 routing,
    combine weights and the balance loss. This *is* the sharding step: it
    decides which tokens go to which core.
  - Core e receives the tokens routed to expert e (gathered, transposed to
    [D, cap], zero-padded to a common capacity), plus expert e's weights
    pre-transposed on host so all matmul operands are K-major.
  - Device computes  y = (relu(x @ Wfc^T)^2 @ Wproj^T) * combine_weight
    with float32r matmuls (full fp32 data, 1 cycle/row at N>=256).
  - Host scatter-adds the per-expert outputs back into [N, D].

All shapes hardcoded for B=2, T=2048, D=768, H=3072, E=8, top-2.
"""

import os
import sys

for _p in ("/opt/trn_rl_repo", "/root/.axon_site/_ro/trn_rl_repo"):
    if os.path.isdir(_p) and _p not in sys.path:
        sys.path.append(_p)

from contextlib import ExitStack

import numpy as np

import concourse.bass as bass
import concourse.tile as tile
from concourse import bacc
from concourse import mybir
from concourse.bass_utils import run_bass_kernel_spmd

TOPK = 2
D, H, E = 768, 3072, 8
P = 128
KD, KH = D // P, H // P  # 6, 24
CHUNK = 256  # tokens per chunk (2 PSUM c-subtiles of 128)
F32 = mybir.dt.float32
F32R = mybir.dt.float32r
ACT = mybir.ActivationFunctionType


def _r(ap):
    # float32r: same fp32 bytes, PE streams 4B/cycle -> 4x faster than float32
    return ap.bitcast(F32R)


def build_nc(cap: int, reps: int = 1, loop_iters: int | None = None) -> bass.Bass:
    """One expert's MLP over `cap` tokens (zero-padded).

    Inputs (per core): xt [D, cap] = tokens^T; wfct [D, H] = Wfc^T;
    wprojt [H, D] = Wproj^T; scale [P, cap//P] = combine weights, token-major
    (token t = cs*128 + p lives at scale[p, cs]).
    Output: y [cap, D], already scaled by the combine weight.
    """
    assert cap % CHUNK == 0
    nc = bacc.Bacc(None, target_bir_lowering=False)
    xt = nc.dram_tensor("xt", [D, cap], F32R, kind="ExternalInput")
    wfct = nc.dram_tensor("wfct", [D, H], F32R, kind="ExternalInput")
    wprojt = nc.dram_tensor("wprojt", [H, D], F32R, kind="ExternalInput")
    scale = nc.dram_tensor("scale", [P, cap // P], F32, kind="ExternalInput")
    y = nc.dram_tensor("y", [cap, D], F32, kind="ExternalOutput")

    xt_r = xt.rearrange("(kd p) c -> p kd c", p=P)  # [128, 6, cap]
    wfct_r = wfct.rearrange("(kd p) h -> p kd h", p=P)  # [128, 6, 3072]
    wprojt_r = wprojt.rearrange("(kh p) d -> p kh d", p=P)  # [128, 24, 768]
    y_r = y.rearrange("(ct p) d -> p ct d", p=P)  # [128, cap//128, 768]

    nchunks = cap // CHUNK
    NSUB = CHUNK // P  # 2

    with ExitStack() as ctx:
        tc = ctx.enter_context(tile.TileContext(nc))
        wpool = ctx.enter_context(tc.tile_pool(name="w", bufs=1))
        xpool = ctx.enter_context(tc.tile_pool(name="x", bufs=2))
        rpool = ctx.enter_context(tc.tile_pool(name="r", bufs=4))
        apool = ctx.enter_context(tc.tile_pool(name="a", bufs=6))
        ypool = ctx.enter_context(tc.tile_pool(name="yo", bufs=3))
        ps1pool = ctx.enter_context(tc.tile_pool(name="ps1", bufs=2, space="PSUM"))
        ps2pool = ctx.enter_context(tc.tile_pool(name="ps2", bufs=1, space="PSUM"))

        # Weights resident in SBUF for the whole kernel (~144 KB/partition).
        # DMAs emitted in first-consumption order so chunk-0 compute starts
        # as soon as wfct block 0 lands and never starves on wproj tiles:
        # wfct h-block 0 -> wproj 0..5 -> wfct hb1 -> wproj 6..11 -> ...
        wfct_sb = []
        for k in range(KD):
            t = wpool.tile([P, H], F32R, name=f"wfc{k}", tag=f"wfc{k}")
            wfct_sb.append(t)
        wprojt_sb = []
        for k in range(KH):
            t = wpool.tile([P, D], F32R, name=f"wpj{k}", tag=f"wpj{k}")
            wprojt_sb.append(t)
        scale_sb = wpool.tile([P, cap // P], F32, name="scale", tag="scale")
        nc.sync.dma_start(out=scale_sb, in_=scale[:, :])
        HBLK = H // 4
        for hb in range(4):
            for k in range(KD):
                nc.sync.dma_start(
                    out=wfct_sb[k][:, hb * HBLK : (hb + 1) * HBLK],
                    in_=wfct_r[:, k, hb * HBLK : (hb + 1) * HBLK],
                )
            for k in range(hb * KD, (hb + 1) * KD):
                nc.sync.dma_start(out=wprojt_sb[k], in_=wprojt_r[:, k, :])
        for k in range(4 * KD, KH):
            nc.sync.dma_start(out=wprojt_sb[k], in_=wprojt_r[:, k, :])

        def full_body():
          for _rep in range(reps):
            body_once()

        def body_once():
          for ci in range(nchunks):
            xt_sb = xpool.tile([P, KD, CHUNK], F32R, name="xt", tag="xt")
            nc.scalar.dma_start(
                out=xt_sb, in_=xt_r[:, :, ci * CHUNK : (ci + 1) * CHUNK]
            )

            # mm2 accumulators for this chunk: [128 tokens, 768] each (2 banks)
            ps2_tiles = [
                ps2pool.tile([P, D], F32, name=f"ps2_{cs}", tag=f"ps2_{cs}") for cs in range(NSUB)
            ]
            a_tiles = [None] * KH

            def mm1_step(hk, xt_sb=xt_sb, a_tiles=a_tiles):
                # A^T[hk] = relu(Wfc x)^2 for 128 h-rows x CHUNK tokens
                ps1 = ps1pool.tile([P, CHUNK], F32, name="ps1", tag="ps1")
                for dk in range(KD):
                    nc.tensor.matmul(
                        ps1,
                        lhsT=wfct_sb[dk][:, hk * P : (hk + 1) * P],
                        rhs=xt_sb[:, dk, :],
                        start=(dk == 0),
                        stop=(dk == KD - 1),
                    )
                r_sb = rpool.tile([P, CHUNK], F32, name="r", tag="r")
                nc.scalar.activation(out=r_sb, in_=ps1, func=ACT.Relu)
                a_sb = apool.tile([P, CHUNK], F32R, name="a", tag="a")
                nc.vector.tensor_mul(a_sb, r_sb, r_sb)
                a_tiles[hk] = a_sb

            def mm2_step(hk, ps2_tiles=ps2_tiles, a_tiles=a_tiles):
                for cs in range(NSUB):
                    a = a_tiles[hk][:, cs * P : (cs + 1) * P]
                    nc.tensor.matmul(
                        ps2_tiles[cs][:, 0:512],
                        lhsT=a,
                        rhs=wprojt_sb[hk][:, 0:512],
                        start=(hk == 0),
                        stop=(hk == KH - 1),
                    )
                    nc.tensor.matmul(
                        ps2_tiles[cs][:, 512:768],
                        lhsT=a,
                        rhs=wprojt_sb[hk][:, 512:768],
                        start=(hk == 0),
                        stop=(hk == KH - 1),
                    )

            # software pipeline: mm2 consumes a-tiles PD steps behind mm1 so
            # the PE never stalls on the relu->square (ACT->DVE) chain
            PD = 2
            for hk in range(PD):
                mm1_step(hk)
            for hk in range(PD, KH):
                mm1_step(hk)
                mm2_step(hk - PD)
            for hk in range(KH - PD, KH):
                mm2_step(hk)

            for cs in range(NSUB):
                ct = ci * NSUB + cs
                y_sb = ypool.tile([P, D], F32, name="y", tag="y")
                nc.vector.tensor_scalar_mul(
                    y_sb, ps2_tiles[cs], scale_sb[:, ct : ct + 1]
                )
                nc.scalar.dma_start(out=y_r[:, ct, :], in_=y_sb)

        if loop_iters is None:
            full_body()
        else:
            with tc.For_i(0, loop_iters, 1):
                full_body()

    nc.finalize()
    return nc


def route(x, gate_w):
    """Host-side gate: returns per-expert token indices + combine weights,
    and the balance loss (matches reference.py's math)."""
    N = x.shape[0]
    logits = x @ gate_w.T  # [N, E]
    m = logits.max(axis=1, keepdims=True)
    p = np.exp(logits - m)
    probs = p / p.sum(axis=1, keepdims=True)  # [N, E]
    # top-2, ties -> lower index first (matches jax.lax.top_k)
    idx = np.argsort(-probs, axis=1, kind="stable")[:, :TOPK]  # [N, 2]
    rw = np.take_along_axis(probs, idx, axis=1)
    rw = rw / rw.sum(axis=1, keepdims=True)

    counts = np.zeros(E, np.int64)
    for k in range(TOPK):
        counts += np.bincount(idx[:, k], minlength=E)
    balance = np.float32(
        (probs.mean(axis=0).astype(np.float64) * (counts / N)).sum() * E
    )

    per_expert = []
    for e in range(E):
        sel = idx == e  # [N, 2]
        tok = np.nonzero(sel.any(axis=1))[0]
        # weight for token n at expert e: rw[n, k] where idx[n, k] == e
        w = np.where(idx[tok, 0] == e, rw[tok, 0], rw[tok, 1]).astype(np.float32)
        per_expert.append((tok, w))
    return per_expert, balance


def _prep_core_inputs(xf, w_fc, w_proj, per_expert, cap):
    in_maps = []
    for e in range(E):
        tok, w = per_expert[e]
        cnt = len(tok)
        xt = np.zeros((D, cap), np.float32)
        xt[:, :cnt] = xf[tok].T
        scale = np.zeros((P, cap // P), np.float32)
        sflat = np.zeros(cap, np.float32)
        sflat[:cnt] = w
        scale[:, :] = sflat.reshape(cap // P, P).T
        in_maps.append(
            {
                "xt": xt,
                "wfct": np.ascontiguousarray(w_fc[e].T),
                "wprojt": np.ascontiguousarray(w_proj[e].T),
                "scale": scale,
            }
        )
    return in_maps


def kernel(x, gate_w, w_fc, w_proj):
    x = np.asarray(x, np.float32)
    gate_w = np.asarray(gate_w, np.float32)
    w_fc = np.asarray(w_fc, np.float32)
    w_proj = np.asarray(w_proj, np.float32)

    B, T, _ = x.shape
    N = B * T
    xf = x.reshape(N, D)

    per_expert, balance = route(xf, gate_w)
    max_cnt = max(len(tok) for tok, _ in per_expert)
    cap = max(CHUNK, ((max_cnt + CHUNK - 1) // CHUNK) * CHUNK)

    nc = build_nc(cap)
    in_maps = _prep_core_inputs(xf, w_fc, w_proj, per_expert, cap)
    results = run_bass_kernel_spmd(nc, in_maps, core_ids=list(range(E))).results

    out = np.zeros((N, D), np.float32)
    for e in range(E):
        tok, _ = per_expert[e]
        out[tok] += results[e]["y"][: len(tok)]
    return out.reshape(B, T, D), balance


# revision 18
# speedup vs baseline: 1.6077x; 1.3912x over previous
"""MoE top-2 routing kernel for 8 Trainium2 NeuronCores.

Strategy (expert parallelism, per sharding hint):
  - Host computes the cheap gate (N x D @ D x E = 25 MFLOP), top-2 routing,
    combine weights and the balance loss. This *is* the sharding step: it
    decides which tokens go to which core.
  - Core e receives the tokens routed to expert e (gathered, transposed to
    [D, cap], zero-padded to a common capacity), plus expert e's weights
    pre-transposed on host so all matmul operands are K-major.
  - Device computes  y = (relu(x @ Wfc^T)^2 @ Wproj^T) * combine_weight
    with float32r matmuls (full fp32 data, 1 cycle/row at N>=256).
  - Host scatter-adds the per-expert outputs back into [N, D].

All shapes hardcoded for B=2, T=2048, D=768, H=3072, E=8, top-2.
"""

import os
import sys

for _p in ("/opt/trn_rl_repo", "/root/.axon_site/_ro/trn_rl_repo"):
    if os.path.isdir(_p) and _p not in sys.path:
        sys.path.append(_p)

from contextlib import ExitStack

import numpy as np

import concourse.bass as bass
import concourse.tile as tile
from concourse import bacc
from concourse import mybir
from concourse.bass_utils import run_bass_kernel_spmd

TOPK = 2
D, H, E = 768, 3072, 8
P = 128
KD, KH = D // P, H // P  # 6, 24
CHUNK = 384  # tokens per main chunk (3 PSUM c-subtile accumulators)
F32 = mybir.dt.float32
F32R = mybir.dt.float32r
ACT = mybir.ActivationFunctionType


def _r(ap):
    # float32r: same fp32 bytes, PE streams 4B/cycle -> 4x faster than float32
    return ap.bitcast(F32R)


def build_nc(cap: int, reps: int = 1, loop_iters: int | None = None) -> bass.Bass:
    """One expert's MLP over `cap` tokens (zero-padded).

    Inputs (per core): xt [D, cap] = tokens^T; wfct [D, H] = Wfc^T;
    wprojt [H, D] = Wproj^T; scale [P, cap//P] = combine weights, token-major
    (token t = cs*128 + p lives at scale[p, cs]).
    Output: y [cap, D], already scaled by the combine weight.
    """
    assert cap % P == 0 and cap % 384 in (0, 256)
    nc = bacc.Bacc(None, target_bir_lowering=False)
    xt = nc.dram_tensor("xt", [D, cap], F32R, kind="ExternalInput")
    wfct = nc.dram_tensor("wfct", [D, H], F32R, kind="ExternalInput")
    wprojt = nc.dram_tensor("wprojt", [H, D], F32R, kind="ExternalInput")
    scale = nc.dram_tensor("scale", [P, cap // P], F32, kind="ExternalInput")
    y = nc.dram_tensor("y", [cap, D], F32, kind="ExternalOutput")

    xt_r = xt.rearrange("(kd p) c -> p kd c", p=P)  # [128, 6, cap]
    wfct_r = wfct.rearrange("(kd p) h -> p kd h", p=P)  # [128, 6, 3072]
    wprojt_r = wprojt.rearrange("(kh p) d -> p kh d", p=P)  # [128, 24, 768]
    y_r = y.rearrange("(ct p) d -> p ct d", p=P)  # [128, cap//128, 768]

    nchunks = cap // CHUNK
    NSUB = CHUNK // P  # 2

    with ExitStack() as ctx:
        tc = ctx.enter_context(tile.TileContext(nc))
        wpool = ctx.enter_context(tc.tile_pool(name="w", bufs=1))
        xpool = ctx.enter_context(tc.tile_pool(name="x", bufs=2))
        rpool = ctx.enter_context(tc.tile_pool(name="r", bufs=2))
        apool = ctx.enter_context(tc.tile_pool(name="a", bufs=4))
        ypool = ctx.enter_context(tc.tile_pool(name="yo", bufs=3))
        ps1pool = ctx.enter_context(tc.tile_pool(name="ps1", bufs=2, space="PSUM"))
        ps2pool = ctx.enter_context(tc.tile_pool(name="ps2", bufs=1, space="PSUM"))

        # Weights resident in SBUF for the whole kernel (~144 KB/partition).
        # DMAs emitted in first-consumption order so chunk-0 compute starts
        # as soon as wfct block 0 lands and never starves on wproj tiles:
        # wfct h-block 0 -> wproj 0..5 -> wfct hb1 -> wproj 6..11 -> ...
        wfct_sb = []
        for k in range(KD):
            t = wpool.tile([P, H], F32R, name=f"wfc{k}", tag=f"wfc{k}")
            wfct_sb.append(t)
        wprojt_sb = []
        for k in range(KH):
            t = wpool.tile([P, D], F32R, name=f"wpj{k}", tag=f"wpj{k}")
            wprojt_sb.append(t)
        scale_sb = wpool.tile([P, cap // P], F32, name="scale", tag="scale")
        nc.sync.dma_start(out=scale_sb, in_=scale[:, :])
        HBLK = H // 4
        for hb in range(4):
            for k in range(KD):
                nc.sync.dma_start(
                    out=wfct_sb[k][:, hb * HBLK : (hb + 1) * HBLK],
                    in_=wfct_r[:, k, hb * HBLK : (hb + 1) * HBLK],
                )
            for k in range(hb * KD, (hb + 1) * KD):
                nc.sync.dma_start(out=wprojt_sb[k], in_=wprojt_r[:, k, :])
        for k in range(4 * KD, KH):
            nc.sync.dma_start(out=wprojt_sb[k], in_=wprojt_r[:, k, :])

        def full_body():
          for _rep in range(reps):
            pos = 0
            while pos < cap:
                ctoks = 384 if cap - pos >= 384 else cap - pos
                chunk_body(pos, ctoks)
                pos += ctoks

        def chunk_body(c0, ctoks):
            nsub = ctoks // P  # 3 (main chunk) or 2 (tail)
            xt_sb = xpool.tile([P, KD, ctoks], F32R, name="xt", tag="xt")
            nc.scalar.dma_start(out=xt_sb, in_=xt_r[:, :, c0 : c0 + ctoks])

            # mm2 accumulators: one [128, 768] (2 PSUM banks) per 128-token
            # subtile; 3 subtiles + double-buffered ps1 = exactly 8 banks.
            ps2_tiles = [
                ps2pool.tile([P, D], F32, name=f"ps2_{cs}", tag=f"ps2_{cs}")
                for cs in range(nsub)
            ]
            a_tiles = [None] * KH

            def mm1_step(hk):
                # A^T[hk] = relu(Wfc x)^2 for 128 h-rows x ctoks tokens
                ps1 = ps1pool.tile([P, ctoks], F32, name="ps1", tag="ps1")
                for dk in range(KD):
                    nc.tensor.matmul(
                        ps1,
                        lhsT=wfct_sb[dk][:, hk * P : (hk + 1) * P],
                        rhs=xt_sb[:, dk, :],
                        start=(dk == 0),
                        stop=(dk == KD - 1),
                    )
                r_sb = rpool.tile([P, ctoks], F32, name="r", tag="r")
                nc.scalar.activation(out=r_sb, in_=ps1, func=ACT.Relu)
                a_sb = apool.tile([P, ctoks], F32R, name="a", tag="a")
                nc.vector.tensor_mul(a_sb, r_sb, r_sb)
                a_tiles[hk] = a_sb

            def mm2_step(hk):
                for cs in range(nsub):
                    a = a_tiles[hk][:, cs * P : (cs + 1) * P]
                    for d0, dn in ((0, 512), (512, 256)):
                        nc.tensor.matmul(
                            ps2_tiles[cs][:, d0 : d0 + dn],
                            lhsT=a,
                            rhs=wprojt_sb[hk][:, d0 : d0 + dn],
                            start=(hk == 0),
                            stop=(hk == KH - 1),
                        )

            # software pipeline: mm2 consumes a-tiles PD steps behind mm1 so
            # the PE never stalls on the relu->square (ACT->DVE) chain
            PD = 2
            for hk in range(PD):
                mm1_step(hk)
            for hk in range(PD, KH):
                mm1_step(hk)
                mm2_step(hk - PD)
            for hk in range(KH - PD, KH):
                mm2_step(hk)

            for cs in range(nsub):
                ct = c0 // P + cs
                y_sb = ypool.tile([P, D], F32, name="y", tag="y")
                nc.vector.tensor_scalar_mul(
                    y_sb, ps2_tiles[cs], scale_sb[:, ct : ct + 1]
                )
                nc.scalar.dma_start(out=y_r[:, ct, :], in_=y_sb)

        if loop_iters is None:
            full_body()
        else:
            with tc.For_i(0, loop_iters, 1):
                full_body()

    nc.finalize()
    return nc


def route(x, gate_w):
    """Host-side gate: returns per-expert token indices + combine weights,
    and the balance loss (matches reference.py's math)."""
    N = x.shape[0]
    logits = x @ gate_w.T  # [N, E]
    m = logits.max(axis=1, keepdims=True)
    p = np.exp(logits - m)
    probs = p / p.sum(axis=1, keepdims=True)  # [N, E]
    # top-2, ties -> lower index first (matches jax.lax.top_k)
    idx = np.argsort(-probs, axis=1, kind="stable")[:, :TOPK]  # [N, 2]
    rw = np.take_along_axis(probs, idx, axis=1)
    rw = rw / rw.sum(axis=1, keepdims=True)

    counts = np.zeros(E, np.int64)
    for k in range(TOPK):
        counts += np.bincount(idx[:, k], minlength=E)
    balance = np.float32(
        (probs.mean(axis=0).astype(np.float64) * (counts / N)).sum() * E
    )

    per_expert = []
    for e in range(E):
        sel = idx == e  # [N, 2]
        tok = np.nonzero(sel.any(axis=1))[0]
        # weight for token n at expert e: rw[n, k] where idx[n, k] == e
        w = np.where(idx[tok, 0] == e, rw[tok, 0], rw[tok, 1]).astype(np.float32)
        per_expert.append((tok, w))
    return per_expert, balance


def _prep_core_inputs(xf, w_fc, w_proj, per_expert, cap):
    in_maps = []
    for e in range(E):
        tok, w = per_expert[e]
        cnt = len(tok)
        xt = np.zeros((D, cap), np.float32)
        xt[:, :cnt] = xf[tok].T
        scale = np.zeros((P, cap // P), np.float32)
        sflat = np.zeros(cap, np.float32)
        sflat[:cnt] = w
        scale[:, :] = sflat.reshape(cap // P, P).T
        in_maps.append(
            {
                "xt": xt,
                "wfct": np.ascontiguousarray(w_fc[e].T),
                "wprojt": np.ascontiguousarray(w_proj[e].T),
                "scale": scale,
            }
        )
    return in_maps


def kernel(x, gate_w, w_fc, w_proj):
    x = np.asarray(x, np.float32)
    gate_w = np.asarray(gate_w, np.float32)
    w_fc = np.asarray(w_fc, np.float32)
    w_proj = np.asarray(w_proj, np.float32)

    B, T, _ = x.shape
    N = B * T
    xf = x.reshape(N, D)

    per_expert, balance = route(xf, gate_w)
    max_cnt = max(len(tok) for tok, _ in per_expert)
    cap = max(256, ((max_cnt + P - 1) // P) * P)
    if cap % 384 == 128:  # avoid a 128-token tail (slow fp32r below N=256)
        cap += 128

    nc = build_nc(cap)
    in_maps = _prep_core_inputs(xf, w_fc, w_proj, per_expert, cap)
    results = run_bass_kernel_spmd(nc, in_maps, core_ids=list(range(E))).results

    out = np.zeros((N, D), np.float32)
    for e in range(E):
        tok, _ = per_expert[e]
        out[tok] += results[e]["y"][: len(tok)]
    return out.reshape(B, T, D), balance
